# revision 40
# baseline (speedup 1.0000x reference)
"""Trainium2 Bass kernel: sparse (top-k) causal attention, data-parallel over batch.

Reference semantics (B=32, H=8, S=512, D=64, k_index=5):
  S_raw = (Q @ K^T) / sqrt(d_k), causal-masked
  P     = softmax(S_raw)
  rows >= k_index: keep only P >= (k_index-th largest of row)
  W     = softmax(P');  W[row 0] = 0;  out = W @ V

On-chip identities (per row):
  - no max-subtraction needed (scores ~ N(0,1))
  - top-k threshold via DVE top-8 in the exp-domain (softmax is monotone)
  - W = (E >= thr) * exp(E/Z) via one fused scalar_tensor_tensor with
    accumulated row-sum Z2; rows < k_index pass everything (thr=-1) and the
    causal-masked cols contribute exp(0)=1, matching the reference; their
    uniform tail beyond the causal tile adds (S-128) to Z2 and a rank-1
    ones @ V term to the output; row 0 is zeroed via its 1/Z2 scale.

Performance structure (each core runs 32 heads x 4 causal q-tiles):
  - QK^T as two accumulating fp16 matmuls: [qh;ql]@[kh;kh] (contraction
    128) + qh@kl, where q=qh+ql, k=kh+kl are fp16 splits -> float32-level
    score accuracy at full PE rate (native fp32 matmul is 1/8 rate)
  - causal mask added to the diagonal block via an identity@mask matmul
    accumulated into the same PSUM group
  - W^T for the second matmul via per-chunk PE transposes + one merged
    PSUM->SBUF evict copy (xbar DMA transpose measured far slower on the
    critical path)
  - heads emitted pairwise interleaved; the Tile scheduler pipelines
    across iterations

Sharding: batch 32 -> 4 per core across 8 cores; each (b,h) independent.
Host packs [qh;ql;kh;kh;kl] pre-transposed as one fp16 [.., 5D, S] tensor
and V as bf16; output is f32.
"""

import math

import numpy as np
import ml_dtypes

import concourse.bass as bass
import concourse.bacc as bacc
import concourse.mybir as mybir
import concourse.tile as tile
from concourse.bass_utils import run_bass_kernel_spmd
from concourse.masks import make_causal_mask, make_identity

N_CORES = 8
F32 = mybir.dt.float32
BF16 = mybir.dt.bfloat16

# test.py hooks
TRACE = False
LAST_RESULT = None
BH_OVERRIDE = None  # dev only: limit (b,h) pairs per core
# matmul1 runs as 3 accumulating fp16 matmuls (qh@kh + qh@kl + ql@kh) where
# q = qh + ql, k = kh + kl are fp16 splits: ~2^-21 relative score error
# (float32-rounding level) at full PE rate (fp32 matmul runs at 1/8 rate).
QK_DTYPE = mybir.dt.float16
MM1_F32 = False  # True: exact fp32 QK matmul (1/8 PE rate) instead of fp16 splits

_NC_CACHE = {}


def _build(bh_count: int, S: int, D: int, d_k: int, k_index: int) -> bass.Bass:
    P = 128
    NT = S // P
    KI = k_index
    NEG = -1.0e5
    scale = 1.0 / math.sqrt(float(d_k))
    assert 1 <= KI <= 8 and S % P == 0 and D <= P

    nc = bacc.Bacc("TRN2", target_bir_lowering=False, debug=False)
    qk_dt = F32 if MM1_F32 else QK_DTYPE
    qk_rows = 2 * D if MM1_F32 else 5 * D
    qkt = nc.declare_dram_parameter("qkt", [bh_count, qk_rows, S], qk_dt, isOutput=False)
    vb = nc.declare_dram_parameter("vb", [bh_count, S, D], BF16, isOutput=False)
    out = nc.declare_dram_parameter("out", [bh_count, S, D], F32, isOutput=True)

    NI = bh_count * NT  # total iterations, i -> (bh=i//NT, t=i%NT)

    with tile.TileContext(nc) as tc:
        with (
            tc.tile_pool(name="const", bufs=1) as cpool,
            tc.tile_pool(name="inp", bufs=5) as ipool,
            tc.tile_pool(name="big", bufs=6) as bpool,
            tc.tile_pool(name="wbuf", bufs=5) as wpool,
            tc.tile_pool(name="wt", bufs=6) as wtpool,
            tc.tile_pool(name="stat", bufs=24) as spool,
            tc.tile_pool(name="obuf", bufs=5) as opool,
            tc.tile_pool(name="ps_s", bufs=4, space="PSUM") as ps_s,
            tc.tile_pool(name="ps_o", bufs=2, space="PSUM") as ps_o,
            tc.tile_pool(name="ps_wt", bufs=2, space="PSUM") as ps_wt,
        ):
            # constants
            mask_f = cpool.tile([P, P], F32)
            make_causal_mask(nc, mask_f[:, :], mask_val=NEG)
            mask_b = cpool.tile([P, P], BF16)
            nc.vector.tensor_copy(mask_b[:, :], mask_f[:, :])
            ident_f = cpool.tile([P, P], F32)
            make_identity(nc, ident_f[:, :])
            ident_b = cpool.tile([P, P], BF16)
            nc.vector.tensor_copy(ident_b[:, :], ident_f[:, :])
            ones_k = cpool.tile([P, KI], BF16)
            nc.vector.memset(ones_k[:, :], 1.0)

            st = {}  # per-iteration tile state
            bh_state = {}  # per-head tiles (qk, v, o_all)

            def s_dma(i):
                bh, t = divmod(i, NT)
                if t:
                    return
                if MM1_F32:
                    qk_s = ipool.tile([2 * D, S], qk_dt, tag="qk", name=f"qk_{bh}")
                    nc.gpsimd.dma_start(qk_s[:, :], qkt[bh])
                    qk_s = (qk_s,)
                else:
                    t_a = ipool.tile([2 * D, S], qk_dt, tag="qka", name=f"qka_{bh}")
                    nc.gpsimd.dma_start(t_a[:, :], qkt[bh, 0 : 2 * D, :])
                    t_b = ipool.tile([2 * D, S], qk_dt, tag="qkb", name=f"qkb_{bh}")
                    nc.gpsimd.dma_start(t_b[:, :], qkt[bh, 2 * D : 4 * D, :])
                    t_c = ipool.tile([D, S], qk_dt, tag="qkc", name=f"qkc_{bh}")
                    nc.gpsimd.dma_start(t_c[:, :], qkt[bh, 4 * D : 5 * D, :])
                    qk_s = (t_a, t_b, t_c)
                v_s = ipool.tile([P, NT, D], BF16, tag="v", name=f"v_{bh}")
                nc.gpsimd.dma_start(
                    v_s[:, :, :], vb[bh].rearrange("(c p) d -> p c d", p=P)
                )
                o_all = opool.tile([P, NT, D], F32, tag="o_all", name=f"oall_{bh}")
                bh_state[bh] = (qk_s, v_s, o_all)

            def s_mm1(i):
                bh, t = divmod(i, NT)
                C = P * (t + 1)
                qk_s = bh_state[bh][0]
                s_ps = ps_s.tile([P, S], F32, tag="s", name=f"sps_{i}")
                if MM1_F32:
                    qa = qk_s[0]
                    nc.tensor.matmul(
                        s_ps[:, :C], lhsT=qa[0:D, bass.ts(t, P)],
                        rhs=qa[D : 2 * D, :C], start=True, stop=False)
                else:
                    t_a, t_b, t_c = qk_s
                    # [qh;ql] @ [kh;kh] (contraction 128) + qh @ kl
                    nc.tensor.matmul(
                        s_ps[:, :C], lhsT=t_a[:, bass.ts(t, P)],
                        rhs=t_b[:, :C], start=True, stop=False)
                    nc.tensor.matmul(
                        s_ps[:, :C], lhsT=t_a[0:D, bass.ts(t, P)],
                        rhs=t_c[:, :C], start=False, stop=False)
                nc.tensor.matmul(
                    s_ps[:, bass.ts(t, P)],
                    lhsT=ident_b[:, :],
                    rhs=mask_b[:, :],
                    start=False,
                    stop=True,
                )
                st[i] = {"s_ps": s_ps}

            def s_exp1(i):
                bh, t = divmod(i, NT)
                C = P * (t + 1)
                d = st[i]
                e_s = bpool.tile([P, S], F32, tag="e", name=f"e_{i}")
                z = spool.tile([P, 1], F32, tag="z", name=f"z_{i}")
                nc.scalar.activation(
                    e_s[:, :C],
                    d["s_ps"][:, :C],
                    mybir.ActivationFunctionType.Exp,
                    scale=scale,
                    accum_out=z[:, :],
                )
                d["e"], d["z"] = e_s, z

            def s_top8(i):
                bh, t = divmod(i, NT)
                C = P * (t + 1)
                d = st[i]
                top8 = spool.tile([P, 8], F32, tag="top8", name=f"top8_{i}")
                nc.vector.max(out=top8[:, :], in_=d["e"][:, :C])
                if t == 0:
                    nc.vector.memset(top8[0:KI, KI - 1 : KI], -1.0)
                rz = spool.tile([P, 1], F32, tag="rz", name=f"rz_{i}")
                nc.vector.reciprocal(rz[:, :], d["z"][:, :])
                d["top8"], d["rz"] = top8, rz

            def s_exp2(i):
                bh, t = divmod(i, NT)
                C = P * (t + 1)
                d = st[i]
                u_s = bpool.tile([P, S], F32, tag="u", name=f"u_{i}")
                nc.scalar.activation(
                    u_s[:, :C],
                    d["e"][:, :C],
                    mybir.ActivationFunctionType.Exp,
                    scale=d["rz"][:, 0:1],
                )
                d["u"] = u_s

            def s_stt(i):
                bh, t = divmod(i, NT)
                C = P * (t + 1)
                d = st[i]
                w_s = wpool.tile([P, S], BF16, tag="w", name=f"w_{i}")
                z2 = spool.tile([P, 1], F32, tag="z2", name=f"z2_{i}")
                nc.vector.scalar_tensor_tensor(
                    out=w_s[:, :C],
                    in0=d["e"][:, :C],
                    scalar=d["top8"][:, KI - 1 : KI],
                    in1=d["u"][:, :C],
                    op0=mybir.AluOpType.is_ge,
                    op1=mybir.AluOpType.mult,
                    accum_out=z2[:, :],
                )
                if t == 0:
                    nc.vector.tensor_scalar_add(z2[0:KI, :], z2[0:KI, :], float(S - P))
                rz2 = spool.tile([P, 1], F32, tag="rz2", name=f"rz2_{i}")
                nc.vector.reciprocal(rz2[:, :], z2[:, :])
                if t == 0:
                    nc.vector.memset(rz2[0:1, :], 0.0)
                d["w"], d["rz2"] = w_s, rz2

            def s_tr(i):
                bh, t = divmod(i, NT)
                C = P * (t + 1)
                d = st[i]
                wt_s = wtpool.tile([P, NT, P], BF16, tag="wt", name=f"wt_{i}")
                wtp = ps_wt.tile([P, NT, P], BF16, tag="wtp", name=f"wtp_{i}")
                for c in range(t + 1):
                    nc.tensor.transpose(
                        wtp[:, c, :], d["w"][:, bass.ts(c, P)], ident_b[:, :]
                    )
                if i % 2 == 0:
                    nc.vector.tensor_copy(wt_s[:, 0 : t + 1, :], wtp[:, 0 : t + 1, :])
                else:
                    nc.scalar.copy(wt_s[:, 0 : t + 1, :], wtp[:, 0 : t + 1, :])
                d["wt"] = wt_s

            def s_mm2(i):
                bh, t = divmod(i, NT)
                d = st[i]
                v_s = bh_state[bh][1]
                o_ps = ps_o.tile([P, D], F32, tag="o", name=f"ops_{i}")
                for c in range(t + 1):
                    nc.tensor.matmul(
                        o_ps[:, :],
                        lhsT=d["wt"][:, c, :],
                        rhs=v_s[:, c, :],
                        start=(c == 0),
                        stop=(c == t and t > 0),
                    )
                if t == 0:
                    for c in range(1, NT):
                        nc.tensor.matmul(
                            o_ps[0:KI, :],
                            lhsT=ones_k[:, 0:KI],
                            rhs=v_s[:, c, :],
                            start=False,
                            stop=(c == NT - 1),
                        )
                d["o_ps"] = o_ps

            def s_osc(i):
                bh, t = divmod(i, NT)
                d = st.pop(i)
                o_all = bh_state[bh][2]
                nc.vector.tensor_scalar(
                    out=o_all[:, t, :],
                    in0=d["o_ps"][:, :],
                    scalar1=d["rz2"][:, 0:1],
                    scalar2=None,
                    op0=mybir.AluOpType.mult,
                )
                if t == NT - 1:
                    nc.gpsimd.dma_start(
                        out[bh].rearrange("(c p) d -> p c d", p=P), o_all[:, :, :]
                    )
                    del bh_state[bh]

            body = [s_mm1, s_exp1, s_top8, s_exp2, s_stt, s_tr, s_mm2, s_osc]
            # G=2 head interleave, plain per-iteration emission (the Tile
            # scheduler does its own lookahead; explicit stage skewing
            # measured worse).
            G = 2
            for g0 in range(0, bh_count, G):
                members = list(range(g0, min(g0 + G, bh_count)))
                for bh in members:
                    s_dma(bh * NT)
                for t in range(NT):
                    for bh in members:
                        i = bh * NT + t
                        for fn in body:
                            fn(i)
    nc.compile()
    return nc


def _get_nc(bh_count, S, D, d_k, k_index):
    key = (bh_count, S, D, d_k, k_index, str(QK_DTYPE), MM1_F32)
    if key not in _NC_CACHE:
        _NC_CACHE[key] = _build(bh_count, S, D, d_k, k_index)
    return _NC_CACHE[key]


def _numpy_fallback(q, k, v, mask, d_k, k_index):
    """Straight port of the reference for inputs the Bass kernel doesn't
    cover (non-causal mask / incompatible shapes). Slow but correct."""
    NEG = np.float32(-1e32)
    b, h, s, _ = q.shape
    scores = np.einsum("bhqd,bhkd->bhqk", q, k) / np.sqrt(np.float32(d_k))
    scores = np.where(mask == 0, NEG, scores)
    scores = scores - scores.max(axis=-1, keepdims=True)
    e = np.exp(scores)
    scores = e / e.sum(axis=-1, keepdims=True)
    sa = scores[:, :, :k_index, :]
    sb = scores[:, :, k_index:, :].reshape(b * h * (s - k_index), s)
    srt = -np.sort(-sb, axis=-1)
    thr = srt[:, k_index - 1 : k_index]
    sb = np.where(sb - thr >= 0, sb, NEG)
    sb = sb.reshape(b, h, s - k_index, s)
    scores = np.concatenate([sa, sb], axis=2)
    scores = scores - scores.max(axis=-1, keepdims=True)
    e = np.exp(scores)
    scores = e / e.sum(axis=-1, keepdims=True)
    scores[:, :, 0, :] = 0.0
    return np.einsum("bhqk,bhkd->bhqd", scores, v).astype(np.float32)


def _is_causal(mask, S):
    if mask is None:
        return True
    m = np.asarray(mask)
    if m.size != S * S:
        return False
    return bool(np.array_equal(m.reshape(S, S) != 0, np.tril(np.ones((S, S), bool))))


def kernel(q, k, v, mask=None, d_k=None, k_index=None, **_unused):
    global LAST_RESULT
    q = np.asarray(q, dtype=np.float32)
    k = np.asarray(k, dtype=np.float32)
    v = np.asarray(v, dtype=np.float32)
    B, H, S, D = q.shape
    d_k = int(d_k) if d_k is not None else D
    k_index = int(k_index) if k_index is not None else 5

    if (
        B % N_CORES != 0
        or S % 128 != 0
        or D > 128
        or not (1 <= k_index <= 8)
        or not _is_causal(mask, S)
    ):
        mask_np = (
            np.asarray(mask)
            if mask is not None
            else np.tril(np.ones((S, S), np.int32))[None, None]
        )
        return _numpy_fallback(q, k, v, mask_np, d_k, k_index)

    bpc = B // N_CORES
    bh_full = bpc * H
    bh_count = BH_OVERRIDE or bh_full

    qT = np.transpose(q, (0, 1, 3, 2))  # [B, H, D, S]
    kT = np.transpose(k, (0, 1, 3, 2))
    if MM1_F32:
        qkt = np.ascontiguousarray(np.concatenate([qT, kT], axis=2))
    else:
        qh = qT.astype(np.float16)
        ql = (qT - qh.astype(np.float32)).astype(np.float16)
        kh = kT.astype(np.float16)
        kl = (kT - kh.astype(np.float32)).astype(np.float16)
        qkt = np.ascontiguousarray(
            np.concatenate([qh, ql, kh, kh, kl], axis=2)
        )  # [B, H, 5D, S]
    vb = np.ascontiguousarray(v.astype(ml_dtypes.bfloat16))

    nc = _get_nc(bh_count, S, D, d_k, k_index)

    in_maps = []
    for i in range(N_CORES):
        sl = slice(i * bpc, (i + 1) * bpc)
        in_maps.append(
            {
                "qkt": qkt[sl].reshape(bh_full, qkt.shape[2], S)[:bh_count],
                "vb": vb[sl].reshape(bh_full, S, D)[:bh_count],
            }
        )

    res = run_bass_kernel_spmd(
        nc, in_maps, core_ids=list(range(N_CORES)), trace=TRACE
    )
    LAST_RESULT = res

    outs = [
        np.asarray(res.results[i]["out"], dtype=np.float32) for i in range(N_CORES)
    ]
    if bh_count != bh_full:
        outs = [
            np.concatenate(
                [o, np.zeros((bh_full - bh_count, S, D), np.float32)], axis=0
            )
            for o in outs
        ]
    return np.concatenate([o.reshape(bpc, H, S, D) for o in outs], axis=0)


# revision 41
# speedup vs baseline: 1.1999x; 1.1999x over previous
"""Trainium2 Bass kernel: sparse (top-k) causal attention, data-parallel over batch.

Reference semantics (B=32, H=8, S=512, D=64, k_index=5):
  S_raw = (Q @ K^T) / sqrt(d_k), causal-masked
  P     = softmax(S_raw)
  rows >= k_index: keep only P >= (k_index-th largest of row)
  W     = softmax(P');  W[row 0] = 0;  out = W @ V

On-chip identities (per row):
  - no max-subtraction needed (scores ~ N(0,1))
  - top-k threshold via DVE top-8 in the exp-domain (softmax is monotone)
  - W = (E >= thr) * exp(E/Z) via one fused scalar_tensor_tensor with
    accumulated row-sum Z2; rows < k_index pass everything (thr=-1) and the
    causal-masked cols contribute exp(0)=1, matching the reference; their
    uniform tail beyond the causal tile adds (S-128) to Z2 and a rank-1
    ones @ V term to the output; row 0 is zeroed via its 1/Z2 scale.

Performance structure (each core runs 32 heads x 4 causal q-tiles):
  - QK^T as two accumulating fp16 matmuls: [qh;ql]@[kh;kh] (contraction
    128) + qh@kl, where q=qh+ql, k=kh+kl are fp16 splits -> float32-level
    score accuracy at full PE rate (native fp32 matmul is 1/8 rate)
  - causal mask added to the diagonal block via an identity@mask matmul
    accumulated into the same PSUM group
  - W^T for the second matmul via per-chunk PE transposes + one merged
    PSUM->SBUF evict copy (xbar DMA transpose measured far slower on the
    critical path)
  - heads emitted pairwise interleaved; the Tile scheduler pipelines
    across iterations

Sharding: batch 32 -> 4 per core across 8 cores; each (b,h) independent.
Host packs [qh;ql;kh;kh;kl] pre-transposed as one fp16 [.., 5D, S] tensor
and V as bf16; output is f32.
"""

import math

import numpy as np
import ml_dtypes

import concourse.bass as bass
import concourse.bacc as bacc
import concourse.mybir as mybir
import concourse.tile as tile
from concourse.bass_utils import run_bass_kernel_spmd
from concourse.masks import make_causal_mask, make_identity

N_CORES = 8
F32 = mybir.dt.float32
BF16 = mybir.dt.bfloat16

# test.py hooks
TRACE = False
LAST_RESULT = None
BH_OVERRIDE = None  # dev only: limit (b,h) pairs per core
# matmul1 runs as 3 accumulating fp16 matmuls (qh@kh + qh@kl + ql@kh) where
# q = qh + ql, k = kh + kl are fp16 splits: ~2^-21 relative score error
# (float32-rounding level) at full PE rate (fp32 matmul runs at 1/8 rate).
QK_DTYPE = mybir.dt.float16
MM1_F32 = False  # True: exact fp32 QK matmul (1/8 PE rate) instead of fp16 splits

_NC_CACHE = {}


def _build(bh_count: int, S: int, D: int, d_k: int, k_index: int) -> bass.Bass:
    P = 128
    NT = S // P
    KI = k_index
    NEG = -1.0e5
    scale = 1.0 / math.sqrt(float(d_k))
    assert 1 <= KI <= 8 and S % P == 0 and D <= P

    nc = bacc.Bacc("TRN2", target_bir_lowering=False, debug=False)
    qk_dt = F32 if MM1_F32 else QK_DTYPE
    qk_rows = 2 * D if MM1_F32 else 5 * D
    qkt = nc.declare_dram_parameter("qkt", [bh_count, qk_rows, S], qk_dt, isOutput=False)
    vb = nc.declare_dram_parameter("vb", [bh_count, S, D], BF16, isOutput=False)
    out = nc.declare_dram_parameter("out", [bh_count, S, D], F32, isOutput=True)

    NI = bh_count * NT  # total iterations, i -> (bh=i//NT, t=i%NT)

    with tile.TileContext(nc) as tc:
        with (
            tc.tile_pool(name="const", bufs=1) as cpool,
            tc.tile_pool(name="inp", bufs=5) as ipool,
            tc.tile_pool(name="big", bufs=6) as bpool,
            tc.tile_pool(name="wbuf", bufs=5) as wpool,
            tc.tile_pool(name="wt", bufs=6) as wtpool,
            tc.tile_pool(name="stat", bufs=24) as spool,
            tc.tile_pool(name="obuf", bufs=5) as opool,
            tc.tile_pool(name="ps_s", bufs=4, space="PSUM") as ps_s,
            tc.tile_pool(name="ps_o", bufs=2, space="PSUM") as ps_o,
            tc.tile_pool(name="ps_wt", bufs=2, space="PSUM") as ps_wt,
        ):
            # constants
            mask_f = cpool.tile([P, P], F32)
            make_causal_mask(nc, mask_f[:, :], mask_val=NEG)
            mask_b = cpool.tile([P, P], BF16)
            nc.vector.tensor_copy(mask_b[:, :], mask_f[:, :])
            ident_f = cpool.tile([P, P], F32)
            make_identity(nc, ident_f[:, :])
            ident_b = cpool.tile([P, P], BF16)
            nc.vector.tensor_copy(ident_b[:, :], ident_f[:, :])
            ones_k = cpool.tile([P, KI], BF16)
            nc.vector.memset(ones_k[:, :], 1.0)

            st = {}  # per-iteration tile state
            bh_state = {}  # per-head tiles (qk, v, o_all)

            def s_dma(i):
                bh, t = divmod(i, NT)
                if t:
                    return
                if MM1_F32:
                    qk_s = ipool.tile([2 * D, S], qk_dt, tag="qk", name=f"qk_{bh}")
                    nc.gpsimd.dma_start(qk_s[:, :], qkt[bh])
                    qk_s = (qk_s,)
                else:
                    t_a = ipool.tile([2 * D, S], qk_dt, tag="qka", name=f"qka_{bh}")
                    nc.gpsimd.dma_start(t_a[:, :], qkt[bh, 0 : 2 * D, :])
                    t_b = ipool.tile([2 * D, S], qk_dt, tag="qkb", name=f"qkb_{bh}")
                    nc.gpsimd.dma_start(t_b[:, :], qkt[bh, 2 * D : 4 * D, :])
                    t_c = ipool.tile([D, S], qk_dt, tag="qkc", name=f"qkc_{bh}")
                    nc.gpsimd.dma_start(t_c[:, :], qkt[bh, 4 * D : 5 * D, :])
                    qk_s = (t_a, t_b, t_c)
                v_s = ipool.tile([P, NT, D], BF16, tag="v", name=f"v_{bh}")
                nc.sync.dma_start(
                    v_s[:, :, :], vb[bh].rearrange("(c p) d -> p c d", p=P)
                )
                o_all = opool.tile([P, NT, D], F32, tag="o_all", name=f"oall_{bh}")
                bh_state[bh] = (qk_s, v_s, o_all)

            def s_mm1(i):
                bh, t = divmod(i, NT)
                C = P * (t + 1)
                qk_s = bh_state[bh][0]
                s_ps = ps_s.tile([P, S], F32, tag="s", name=f"sps_{i}")
                if MM1_F32:
                    qa = qk_s[0]
                    nc.tensor.matmul(
                        s_ps[:, :C], lhsT=qa[0:D, bass.ts(t, P)],
                        rhs=qa[D : 2 * D, :C], start=True, stop=False)
                else:
                    t_a, t_b, t_c = qk_s
                    # [qh;ql] @ [kh;kh] (contraction 128) + qh @ kl
                    nc.tensor.matmul(
                        s_ps[:, :C], lhsT=t_a[:, bass.ts(t, P)],
                        rhs=t_b[:, :C], start=True, stop=False)
                    nc.tensor.matmul(
                        s_ps[:, :C], lhsT=t_a[0:D, bass.ts(t, P)],
                        rhs=t_c[:, :C], start=False, stop=False)
                nc.tensor.matmul(
                    s_ps[:, bass.ts(t, P)],
                    lhsT=ident_b[:, :],
                    rhs=mask_b[:, :],
                    start=False,
                    stop=True,
                )
                st[i] = {"s_ps": s_ps}

            def s_exp1(i):
                bh, t = divmod(i, NT)
                C = P * (t + 1)
                d = st[i]
                e_s = bpool.tile([P, S], F32, tag="e", name=f"e_{i}")
                z = spool.tile([P, 1], F32, tag="z", name=f"z_{i}")
                nc.scalar.activation(
                    e_s[:, :C],
                    d["s_ps"][:, :C],
                    mybir.ActivationFunctionType.Exp,
                    scale=scale,
                    accum_out=z[:, :],
                )
                d["e"], d["z"] = e_s, z

            def s_top8(i):
                bh, t = divmod(i, NT)
                C = P * (t + 1)
                d = st[i]
                top8 = spool.tile([P, 8], F32, tag="top8", name=f"top8_{i}")
                nc.vector.max(out=top8[:, :], in_=d["e"][:, :C])
                if t == 0:
                    nc.vector.memset(top8[0:KI, KI - 1 : KI], -1.0)
                rz = spool.tile([P, 1], F32, tag="rz", name=f"rz_{i}")
                nc.vector.reciprocal(rz[:, :], d["z"][:, :])
                d["top8"], d["rz"] = top8, rz

            def s_exp2(i):
                bh, t = divmod(i, NT)
                C = P * (t + 1)
                d = st[i]
                u_s = bpool.tile([P, S], F32, tag="u", name=f"u_{i}")
                nc.scalar.activation(
                    u_s[:, :C],
                    d["e"][:, :C],
                    mybir.ActivationFunctionType.Exp,
                    scale=d["rz"][:, 0:1],
                )
                d["u"] = u_s

            def s_stt(i):
                bh, t = divmod(i, NT)
                C = P * (t + 1)
                d = st[i]
                w_s = wpool.tile([P, S], BF16, tag="w", name=f"w_{i}")
                z2 = spool.tile([P, 1], F32, tag="z2", name=f"z2_{i}")
                nc.vector.scalar_tensor_tensor(
                    out=w_s[:, :C],
                    in0=d["e"][:, :C],
                    scalar=d["top8"][:, KI - 1 : KI],
                    in1=d["u"][:, :C],
                    op0=mybir.AluOpType.is_ge,
                    op1=mybir.AluOpType.mult,
                    accum_out=z2[:, :],
                )
                if t == 0:
                    nc.vector.tensor_scalar_add(z2[0:KI, :], z2[0:KI, :], float(S - P))
                rz2 = spool.tile([P, 1], F32, tag="rz2", name=f"rz2_{i}")
                nc.vector.reciprocal(rz2[:, :], z2[:, :])
                if t == 0:
                    nc.vector.memset(rz2[0:1, :], 0.0)
                d["w"], d["rz2"] = w_s, rz2

            def s_tr(i):
                bh, t = divmod(i, NT)
                C = P * (t + 1)
                d = st[i]
                wt_s = wtpool.tile([P, NT, P], BF16, tag="wt", name=f"wt_{i}")
                wtp = ps_wt.tile([P, NT, P], BF16, tag="wtp", name=f"wtp_{i}")
                for c in range(t + 1):
                    nc.tensor.transpose(
                        wtp[:, c, :], d["w"][:, bass.ts(c, P)], ident_b[:, :]
                    )
                if i % 2 == 0:
                    nc.vector.tensor_copy(wt_s[:, 0 : t + 1, :], wtp[:, 0 : t + 1, :])
                else:
                    nc.scalar.copy(wt_s[:, 0 : t + 1, :], wtp[:, 0 : t + 1, :])
                d["wt"] = wt_s

            def s_mm2(i):
                bh, t = divmod(i, NT)
                d = st[i]
                v_s = bh_state[bh][1]
                o_ps = ps_o.tile([P, D], F32, tag="o", name=f"ops_{i}")
                for c in range(t + 1):
                    nc.tensor.matmul(
                        o_ps[:, :],
                        lhsT=d["wt"][:, c, :],
                        rhs=v_s[:, c, :],
                        start=(c == 0),
                        stop=(c == t and t > 0),
                    )
                if t == 0:
                    for c in range(1, NT):
                        nc.tensor.matmul(
                            o_ps[0:KI, :],
                            lhsT=ones_k[:, 0:KI],
                            rhs=v_s[:, c, :],
                            start=False,
                            stop=(c == NT - 1),
                        )
                d["o_ps"] = o_ps

            def s_osc(i):
                bh, t = divmod(i, NT)
                d = st.pop(i)
                o_all = bh_state[bh][2]
                nc.vector.tensor_scalar(
                    out=o_all[:, t, :],
                    in0=d["o_ps"][:, :],
                    scalar1=d["rz2"][:, 0:1],
                    scalar2=None,
                    op0=mybir.AluOpType.mult,
                )
                if t == NT - 1:
                    nc.sync.dma_start(
                        out[bh].rearrange("(c p) d -> p c d", p=P), o_all[:, :, :]
                    )
                    del bh_state[bh]

            body = [s_mm1, s_exp1, s_top8, s_exp2, s_stt, s_tr, s_mm2, s_osc]
            # G=2 head interleave, plain per-iteration emission (the Tile
            # scheduler does its own lookahead; explicit stage skewing
            # measured worse).
            G = 2
            for g0 in range(0, bh_count, G):
                members = list(range(g0, min(g0 + G, bh_count)))
                for bh in members:
                    s_dma(bh * NT)
                for t in range(NT):
                    for bh in members:
                        i = bh * NT + t
                        for fn in body:
                            fn(i)
    nc.compile()
    return nc


def _get_nc(bh_count, S, D, d_k, k_index):
    key = (bh_count, S, D, d_k, k_index, str(QK_DTYPE), MM1_F32)
    if key not in _NC_CACHE:
        _NC_CACHE[key] = _build(bh_count, S, D, d_k, k_index)
    return _NC_CACHE[key]


def _numpy_fallback(q, k, v, mask, d_k, k_index):
    """Straight port of the reference for inputs the Bass kernel doesn't
    cover (non-causal mask / incompatible shapes). Slow but correct."""
    NEG = np.float32(-1e32)
    b, h, s, _ = q.shape
    scores = np.einsum("bhqd,bhkd->bhqk", q, k) / np.sqrt(np.float32(d_k))
    scores = np.where(mask == 0, NEG, scores)
    scores = scores - scores.max(axis=-1, keepdims=True)
    e = np.exp(scores)
    scores = e / e.sum(axis=-1, keepdims=True)
    sa = scores[:, :, :k_index, :]
    sb = scores[:, :, k_index:, :].reshape(b * h * (s - k_index), s)
    srt = -np.sort(-sb, axis=-1)
    thr = srt[:, k_index - 1 : k_index]
    sb = np.where(sb - thr >= 0, sb, NEG)
    sb = sb.reshape(b, h, s - k_index, s)
    scores = np.concatenate([sa, sb], axis=2)
    scores = scores - scores.max(axis=-1, keepdims=True)
    e = np.exp(scores)
    scores = e / e.sum(axis=-1, keepdims=True)
    scores[:, :, 0, :] = 0.0
    return np.einsum("bhqk,bhkd->bhqd", scores, v).astype(np.float32)


def _is_causal(mask, S):
    if mask is None:
        return True
    m = np.asarray(mask)
    if m.size != S * S:
        return False
    return bool(np.array_equal(m.reshape(S, S) != 0, np.tril(np.ones((S, S), bool))))


def kernel(q, k, v, mask=None, d_k=None, k_index=None, **_unused):
    global LAST_RESULT
    q = np.asarray(q, dtype=np.float32)
    k = np.asarray(k, dtype=np.float32)
    v = np.asarray(v, dtype=np.float32)
    B, H, S, D = q.shape
    d_k = int(d_k) if d_k is not None else D
    k_index = int(k_index) if k_index is not None else 5

    if (
        B % N_CORES != 0
        or S % 128 != 0
        or D > 128
        or not (1 <= k_index <= 8)
        or not _is_causal(mask, S)
    ):
        mask_np = (
            np.asarray(mask)
            if mask is not None
            else np.tril(np.ones((S, S), np.int32))[None, None]
        )
        return _numpy_fallback(q, k, v, mask_np, d_k, k_index)

    bpc = B // N_CORES
    bh_full = bpc * H
    bh_count = BH_OVERRIDE or bh_full

    qT = np.transpose(q, (0, 1, 3, 2))  # [B, H, D, S]
    kT = np.transpose(k, (0, 1, 3, 2))
    if MM1_F32:
        qkt = np.ascontiguousarray(np.concatenate([qT, kT], axis=2))
    else:
        qh = qT.astype(np.float16)
        ql = (qT - qh.astype(np.float32)).astype(np.float16)
        kh = kT.astype(np.float16)
        kl = (kT - kh.astype(np.float32)).astype(np.float16)
        qkt = np.ascontiguousarray(
            np.concatenate([qh, ql, kh, kh, kl], axis=2)
        )  # [B, H, 5D, S]
    vb = np.ascontiguousarray(v.astype(ml_dtypes.bfloat16))

    nc = _get_nc(bh_count, S, D, d_k, k_index)

    in_maps = []
    for i in range(N_CORES):
        sl = slice(i * bpc, (i + 1) * bpc)
        in_maps.append(
            {
                "qkt": qkt[sl].reshape(bh_full, qkt.shape[2], S)[:bh_count],
                "vb": vb[sl].reshape(bh_full, S, D)[:bh_count],
            }
        )

    res = run_bass_kernel_spmd(
        nc, in_maps, core_ids=list(range(N_CORES)), trace=TRACE
    )
    LAST_RESULT = res

    outs = [
        np.asarray(res.results[i]["out"], dtype=np.float32) for i in range(N_CORES)
    ]
    if bh_count != bh_full:
        outs = [
            np.concatenate(
                [o, np.zeros((bh_full - bh_count, S, D), np.float32)], axis=0
            )
            for o in outs
        ]
    return np.concatenate([o.reshape(bpc, H, S, D) for o in outs], axis=0)
